# revision 6
# baseline (speedup 1.0000x reference)
"""Bass/Trainium2 kernel for nn_ContrastiveAlignmentLoss.

reference math (B=256, N=512):
    global_sim = graph.mean(axis=(1, 2))                    # [B]
    sim        = outer(global_sim, global_sim)              # [B, B]
    same       = labels[:, None] == labels[None, :]
    pair_loss  = where(same, relu(0.5 - sim), relu(sim - 0.5))
    loss       = sum(triu(pair_loss, k=1)) / (B*(B-1)/2)

Distribution: data-parallel over B across 8 NeuronCores. Each core
sum-pools its 32 relation graphs (32 MiB/core of HBM reads - the whole
cost), AllGathers the tiny [B] scaled sums, and computes the pairwise
loss replicated. pair_loss is symmetric, so sum over i<j equals
(sum over all i,j - sum over diagonal) / 2, with
    pair(i,j) = relu(d) - same*d,   d = sim - 0.5
    pair(i,i) = relu(0.5 - g_i^2) = -min(g_i^2 - 0.5, 0)

Perf notes:
- the load is shaped around SDMA packet-processing overhead: with 1 MiB
  per-graph tiles every descriptor is an 8 KiB partition run and all 16
  SDMA engines saturate at ~400ns/packet (~303 GB/s aggregate). 4-graph
  tiles ([128, 8192]) make every descriptor a 32 KiB contiguous run,
  lifting per-engine throughput; graphs stay partition-aligned (each
  graph = 32 partitions), so per-graph sums come from a block-indicator
  matmul instead of ones - in graph order, no permutation anywhere.
- tile sizes taper 4,4,4,4,4,4,2,2,1,1,1,1 graphs so the reduces left
  after the last packet are short; the last graph is column-split
  across both rings with halves reduced on ACT+DVE in parallel.
- every auxiliary transfer (labels, consts, warm-up payload) rides the
  gpsimd SWDGE queue so the two HWDGE rings carry nothing but graph
  bytes from the first instruction on.
- engine streams are ordered to avoid in-order stalls: ACT issues each
  scalar-ring DMA before the (cross-ring) reduce that frees the next
  pool slot it will need; DVE (which issues nothing) takes the reduces
  that would otherwise couple a ring to its own data.
- a 32-byte warm-up AllGather fired ~3us in re-synchronizes the ranks
  so the real AllGather's ncfw wakeup + entry barrier cost ~1-2us
  instead of 20-60us.
- pairwise phase: d = INV2*g_i*g_j - 0.5 comes straight out of a K=2
  PE outer product (row1 of lhsT/rhs is a const -0.5/ones pair); DVE
  does same*d and relu(d)-same*d with accum_out. Diagonal terms run on
  ACT (Square then Relu(-x) with accum_out) in parallel.
"""

import numpy as np

import concourse.bacc as bacc
import concourse.mybir as mybir
import concourse.tile as tile
from concourse.bass_utils import run_bass_kernel_spmd

N_CORES = 8
B = 256
N = 512
BS = B // N_CORES          # 32 graphs per core
NN = N * N                 # 262144 elements per graph
P = 128                    # SBUF partitions
MARGIN = 0.5
NUM_PAIRS = B * (B - 1) // 2
INV2 = 1.0 / (float(NN) * float(NN))   # folds the two mean divisions
KK = 0.5 / NUM_PAIRS

# load-tile taper: graphs per tile (sum = 32)
TILE_G = [4, 4, 4, 4, 4, 4, 2, 2, 1, 1, 1, 1]
TILE_BASE = np.cumsum([0] + TILE_G).tolist()   # first graph per tile

# knobs for test.py (harness never touches these)
TRACE = False
TRACE_DIR = None
TRACE_CORES = None
LAST_EXEC_NS = None
LAST_RESULTS = None

_CACHED_NC = None


def build_body(tc, loss_ap, graph_ap, labels_ap, consts_ap):
    """Emit the per-core program. graph_ap: [BS, N, N] f32 shard,
    labels_ap: [1, B] f32 full labels, consts_ap: [P, 6] f32 block
    indicators (blk4 | blk2), loss_ap: [1, 1] f32 out."""
    nc = tc.nc
    f32 = mybir.dt.float32
    X = mybir.AxisListType.X
    ALU = mybir.AluOpType
    Copy = mybir.ActivationFunctionType.Copy

    flat = graph_ap.rearrange("b n m -> (b n m)")

    def gview(t):
        C = TILE_G[t] * 2048
        base = TILE_BASE[t] * NN
        return flat[base : base + P * C].rearrange("(p c) -> p c", c=C)

    with (
        tc.tile_pool(name="io4", bufs=4) as io4,     # [P, 8192] 32K/part
        tc.tile_pool(name="io2", bufs=2) as io2,     # [P, 4096] 16K/part
        tc.tile_pool(name="io1", bufs=3) as io1,     # [P, 2048]  8K/part
        tc.tile_pool(name="acc", bufs=1) as acc,
        tc.tile_pool(name="ps1", bufs=2, space="PSUM") as ps1,
        tc.tile_pool(name="psg", bufs=2, space="PSUM") as psg,
        tc.tile_pool(name="psd", bufs=2, space="PSUM") as psd,
        tc.tile_pool(name="dram", bufs=1, space="DRAM") as dram,
    ):
        # collective buffers (Shared outputs: HBM-HBM AllGather fast path)
        warm_in0 = dram.tile([1, 8], f32, tag="warm_in0")
        warm_out0 = dram.tile([N_CORES, 8], f32, tag="warm_out0", addr_space="Shared")
        cc_in = dram.tile([1, BS], f32, tag="cc_in")
        cc_out = dram.tile([N_CORES, BS], f32, tag="cc_out", addr_space="Shared")

        # ---- gpsimd (SWDGE) aux stream: warm-up CC first, then inputs ----
        nc.gpsimd.dma_start(warm_in0[:], labels_ap[:, 0:8])
        # warm-up collective, fired ~3us in: absorbs the CC entry barrier +
        # ncfw wakeup + rank launch skew so the real AllGather is ~1-2us.
        nc.gpsimd.collective_compute(
            "AllGather",
            ALU.bypass,
            replica_groups=[list(range(N_CORES))],
            ins=[warm_in0[:]],
            outs=[warm_out0[:]],
        )
        lab_row = acc.tile([1, B], f32, tag="lab_row")
        nc.gpsimd.dma_start(lab_row[:], labels_ap)
        lab_cols = []
        for c in range(2):
            lab_col = acc.tile([P, 1], f32, tag=f"lab_col{c}")
            nc.gpsimd.dma_start(lab_col[:], labels_ap[0, c * P : (c + 1) * P])
            lab_cols.append(lab_col)
        blk = acc.tile([P, 6], f32, tag="blk")       # blk4 | blk2
        nc.gpsimd.dma_start(blk[:], consts_ap)
        lb = acc.tile([P, B], f32, tag="lb")
        nc.gpsimd.partition_broadcast(lb[:], lab_row[:])

        # ---- DVE constants + same-label masks (early; waits only on lb) ----
        ones_col = acc.tile([P, 1], f32, tag="ones_col")
        nc.vector.memset(ones_col[:], 1.0)
        # pairwise-phase constant rows (row1 of the K=2 outer product)
        rhs2 = acc.tile([2, B], f32, tag="rhs2")      # row0: INV2*g, row1: 1
        nc.vector.memset(rhs2[:], 1.0)
        combo = acc.tile([2, B], f32, tag="combo")    # row0: raw g, row1: -0.5
        nc.vector.memset(combo[:], -MARGIN)           # lhsT chunks slice this
        diag_bias = acc.tile([1, 1], f32, tag="diag_bias")
        nc.vector.memset(diag_bias[:], MARGIN * KK)
        sames = []
        for c in range(2):
            same = acc.tile([P, B], f32, tag=f"same{c}")
            nc.vector.tensor_scalar(
                same[:], lb[:], lab_cols[c][:], None, ALU.is_equal
            )
            sames.append(same)

        # ---- heavy phase: tapered tiles on both rings ----
        # S columns: tile t -> col t (t = 0..10); tile 11 -> cols 11, 12
        S = acc.tile([P, 13], f32, tag="S")
        SQ = float(np.sqrt(INV2))
        gA = acc.tile([4, 6], f32, tag="gA")
        gB = acc.tile([2, 2], f32, tag="gB")
        gC = acc.tile([1, 2], f32, tag="gC")
        gD = acc.tile([1, 2], f32, tag="gD")
        scrD = acc.tile([1, 2], f32, tag="scrD")

        # per-stage cc_in views, (partition j, col t) -> graph base + G*t + j
        ccvA = cc_in[:, 0:24].rearrange("r (t j) -> (r j) t", j=4)   # [4, 6]
        ccvB = cc_in[:, 24:28].rearrange("r (t j) -> (r j) t", j=2)  # [2, 2]

        pools = {4: io4, 2: io2, 1: io1}

        # Only per-engine emission order matters (engines run in-order;
        # cross-engine sync is via data-dep semaphores). Streams:
        #   sync: T0 T2 T4 T6 T8 T10 T11a stD graw loss
        #   ACT:  T1 T3 red0 T5 red2 T7 T9 red4 red6 red8 T11b red10 red11a ...
        #   DVE:  consts sames red1 red3 red5 sclA red7 sclB red9 sclC red11b sclD ...
        # ACT issues every scalar-ring DMA before the reduce whose pool
        # slot it will need next, so the ring never stalls on compute.
        tl = {}
        for t in range(12):
            tl[t] = pools[TILE_G[t]].tile(
                [P, TILE_G[t] * 2048], f32,
                tag=f"g{TILE_G[t]}tile", name=f"gtile{t}",
            )

        def red_act(t, col=None):
            col = t if col is None else col
            nc.scalar.activation(
                tl[t][:], tl[t][:], Copy, accum_out=S[:, col : col + 1]
            )

        def red_dve(t, col=None):
            col = t if col is None else col
            nc.vector.reduce_sum(S[:, col : col + 1], tl[t][:], axis=X)

        H = 1024                                    # last-graph half cols
        nc.sync.dma_start(tl[0][:], gview(0))       # T0
        nc.scalar.dma_start(tl[1][:], gview(1))     # T1
        nc.scalar.dma_start(tl[3][:], gview(3))     # T3 (slot fresh)
        nc.sync.dma_start(tl[2][:], gview(2))       # T2
        red_act(0)                                  # ACT: t0 (sync ring)
        red_dve(1)                                  # DVE: t1
        nc.sync.dma_start(tl[4][:], gview(4))       # T4 (waits red0)
        nc.scalar.dma_start(tl[5][:], gview(5))     # T5 (waits red1)
        red_act(2)
        red_dve(3)
        nc.sync.dma_start(tl[6][:], gview(6))       # T6
        nc.scalar.dma_start(tl[7][:], gview(7))     # T7
        nc.scalar.dma_start(tl[9][:], gview(9))     # T9 (slot fresh)
        nc.sync.dma_start(tl[8][:], gview(8))       # T8
        nc.sync.dma_start(tl[10][:], gview(10))     # T10
        red_act(4)
        red_dve(5)
        # stage A: graphs 0..23 (blk4 over S cols 0..5)
        psA = psg.tile([4, 6], f32, tag="psst")
        nc.tensor.matmul(psA[:], blk[:, 0:4], S[:, 0:6])
        nc.vector.tensor_scalar(gA[:], psA[:], SQ, None, ALU.mult)
        nc.sync.dma_start(ccvA, gA[:])
        red_act(6)
        red_dve(7)
        # stage B: graphs 24..27 (blk2 over S cols 6..7)
        psB = psg.tile([2, 2], f32, tag="psst")
        nc.tensor.matmul(psB[:], blk[:, 4:6], S[:, 6:8])
        nc.vector.tensor_scalar(gB[:], psB[:], SQ, None, ALU.mult)
        nc.sync.dma_start(ccvB, gB[:])
        red_act(8)
        red_dve(9)
        # stage C: graphs 28, 29 (ones over S cols 8..9)
        psC = psg.tile([1, 2], f32, tag="psst")
        nc.tensor.matmul(psC[:], ones_col[:], S[:, 8:10])
        nc.vector.tensor_scalar(gC[:], psC[:], SQ, None, ALU.mult)
        nc.sync.dma_start(cc_in[:, 28:30], gC[:])
        # last graph (tile 11): halves on both rings, reduced in parallel
        nc.sync.dma_start(tl[11][:, 0:H], gview(11)[:, 0:H])     # T11a
        nc.scalar.dma_start(tl[11][:, H : 2 * H], gview(11)[:, H : 2 * H])
        red_act(10)
        nc.scalar.activation(
            tl[11][:, 0:H], tl[11][:, 0:H], Copy, accum_out=S[:, 11:12]
        )
        nc.vector.reduce_sum(S[:, 12:13], tl[11][:, H : 2 * H], axis=X)
        # stage D: graph 30 (col 10) + graph 31 (cols 11+12 pair-merged)
        psD = psg.tile([1, 3], f32, tag="psst")
        nc.tensor.matmul(psD[:], ones_col[:], S[:, 10:13])
        nc.vector.tensor_scalar(gD[0:1, 0:1], psD[0:1, 0:1], SQ, None, ALU.mult)
        nc.vector.tensor_scalar(
            scrD[:], psD[0:1, 1:3], SQ, None, ALU.mult,
            op1=ALU.add, accum_out=gD[0:1, 1:2],
        )
        nc.sync.dma_start(cc_in[:, 30:32], gD[:])

        # ---- all-gather the [BS] scaled sums -> [B] ----
        nc.gpsimd.collective_compute(
            "AllGather",
            ALU.bypass,
            replica_groups=[list(range(N_CORES))],
            ins=[cc_in[:]],
            outs=[cc_out[:]],
        )

        # gathered pre-scaled sums ghat = sqrt(INV2)*sum, flat [B] in DRAM;
        # two parallel DMAs fill both matmul operand rows, no scale op.
        flatg = cc_out[:].rearrange("r b -> (r b)")
        graw = combo[0:1, :]
        nc.sync.dma_start(graw, flatg[None, :])
        nc.scalar.dma_start(rhs2[0:1, :], flatg[None, :])

        # diagonal terms on ACT: dneg = sum(relu(0.5*KK - gg2)),
        # gg2 = (ghat*sqrt(KK))^2; loss uses -dneg.
        sq = float(np.sqrt(KK))
        gg2 = acc.tile([1, B], f32, tag="gg2")
        nc.scalar.activation(
            gg2[:], graw, mybir.ActivationFunctionType.Square, scale=sq
        )
        dneg = acc.tile([1, 1], f32, tag="dneg")
        relu_tmp = acc.tile([1, B], f32, tag="relu_tmp")
        nc.scalar.activation(
            relu_tmp[:], gg2[:], mybir.ActivationFunctionType.Relu,
            scale=-1.0, bias=diag_bias[:], accum_out=dneg[:],
        )

        # ---- pairwise loss: d straight out of a K=2 PE outer product ----
        CS = acc.tile([P, 2], f32, tag="CS")
        for c in range(2):
            dps = psd.tile([P, B], f32, tag="dps")   # d = INV2*gi*gj - 0.5
            nc.tensor.matmul(dps[:], combo[:, c * P : (c + 1) * P], rhs2[:])
            sd = acc.tile([P, B], f32, tag=f"sd{c}")    # same * d
            nc.vector.tensor_tensor(sd[:], sames[c][:], dps[:], ALU.mult)
            pair = acc.tile([P, B], f32, tag=f"pair{c}")  # relu(d) - sd
            nc.vector.scalar_tensor_tensor(
                pair[:], dps[:], 0.0, sd[:], ALU.max, ALU.subtract,
                accum_out=CS[:, c : c + 1],
            )

        # total = sum all (i,j); loss = KK*total - dneg
        ps_tot = ps1.tile([1, 2], f32, tag="ps_tot")
        nc.tensor.matmul(ps_tot[:], ones_col[:], CS[:])
        tk = acc.tile([1, 2], f32, tag="tk")
        totk = acc.tile([1, 1], f32, tag="totk")
        nc.vector.tensor_scalar(
            tk[:], ps_tot[:], KK, None, ALU.mult, op1=ALU.add,
            accum_out=totk[:],
        )
        res = acc.tile([1, 1], f32, tag="res")
        nc.vector.tensor_tensor(res[:], totk[:], dneg[:], ALU.subtract)
        nc.sync.dma_start(loss_ap, res[:])


def _consts_host():
    """[P, 6] f32 block indicators: blk4 (cols 0-3) | blk2 (cols 4-5)."""
    c = np.zeros((P, 6), dtype=np.float32)
    p = np.arange(P)
    for j in range(4):
        c[p // 32 == j, j] = 1.0
    for j in range(2):
        c[p // 64 == j, 4 + j] = 1.0
    return c


def _build():
    global _CACHED_NC
    if _CACHED_NC is not None:
        return _CACHED_NC
    nc = bacc.Bacc(
        "TRN2", target_bir_lowering=False, debug=False, num_devices=N_CORES
    )
    g_in = nc.dram_tensor(
        "graph", [BS, N, N], mybir.dt.float32, kind="ExternalInput"
    )
    lab_in = nc.dram_tensor(
        "labels_f32", [1, B], mybir.dt.float32, kind="ExternalInput"
    )
    consts_in = nc.dram_tensor(
        "consts", [P, 6], mybir.dt.float32, kind="ExternalInput"
    )
    out = nc.dram_tensor("loss", [1, 1], mybir.dt.float32, kind="ExternalOutput")
    with tile.TileContext(nc) as tc:
        build_body(tc, out.ap(), g_in.ap(), lab_in.ap(), consts_in.ap())
    nc.compile()
    _CACHED_NC = nc
    return nc


def kernel(graph, labels):
    global LAST_EXEC_NS, LAST_RESULTS
    graph = np.ascontiguousarray(np.asarray(graph), dtype=np.float32)
    labels_f32 = np.asarray(labels).astype(np.float32).reshape(1, B)
    assert graph.shape == (B, N, N)
    consts = _consts_host()

    nc = _build()
    in_maps = [
        {
            "graph": graph[c * BS : (c + 1) * BS],
            "labels_f32": labels_f32,
            "consts": consts,
        }
        for c in range(N_CORES)
    ]
    res = run_bass_kernel_spmd(
        nc,
        in_maps,
        core_ids=list(range(N_CORES)),
        trace=TRACE,
        tmpdir=TRACE_DIR,
        trace_cores=TRACE_CORES,
    )
    LAST_RESULTS = res
    LAST_EXEC_NS = res.exec_time_ns
    return np.asarray(res.results[0]["loss"][0, 0], dtype=np.float32)


# revision 8
# speedup vs baseline: 1.0186x; 1.0186x over previous
"""Bass/Trainium2 kernel for nn_ContrastiveAlignmentLoss.

reference math (B=256, N=512):
    global_sim = graph.mean(axis=(1, 2))                    # [B]
    sim        = outer(global_sim, global_sim)              # [B, B]
    same       = labels[:, None] == labels[None, :]
    pair_loss  = where(same, relu(0.5 - sim), relu(sim - 0.5))
    loss       = sum(triu(pair_loss, k=1)) / (B*(B-1)/2)

Distribution: data-parallel over B across 8 NeuronCores. Each core
sum-pools its 32 relation graphs (32 MiB/core of HBM reads - the whole
cost), AllGathers the tiny [B] scaled sums, and computes the pairwise
loss replicated. pair_loss is symmetric, so sum over i<j equals
(sum over all i,j - sum over diagonal) / 2, with
    pair(i,j) = relu(d) - same*d,   d = sim - 0.5
    pair(i,i) = relu(0.5 - g_i^2) = -min(g_i^2 - 0.5, 0)

Perf notes:
- the load is shaped around SDMA packet-processing overhead: with 1 MiB
  per-graph tiles every descriptor is an 8 KiB partition run and all 16
  SDMA engines saturate at ~400ns/packet (~303 GB/s aggregate). 4-graph
  tiles ([128, 8192]) make every descriptor a 32 KiB contiguous run,
  lifting per-engine throughput; graphs stay partition-aligned (each
  graph = 32 partitions), so per-graph sums come from a block-indicator
  matmul instead of ones - in graph order, no permutation anywhere.
- tile sizes taper 4,4,4,4,4,4,2,2,1,1,1,1 graphs so the reduces left
  after the last packet are short; the last graph is column-split
  across both rings with halves reduced on ACT+DVE in parallel.
- every auxiliary transfer (labels, consts, warm-up payload) rides the
  gpsimd SWDGE queue so the two HWDGE rings carry nothing but graph
  bytes from the first instruction on.
- engine streams are ordered to avoid in-order stalls: ACT issues each
  scalar-ring DMA before the (cross-ring) reduce that frees the next
  pool slot it will need; DVE (which issues nothing) takes the reduces
  that would otherwise couple a ring to its own data.
- a 32-byte warm-up AllGather fired ~3us in re-synchronizes the ranks
  so the real AllGather's ncfw wakeup + entry barrier cost ~1-2us
  instead of 20-60us.
- pairwise phase: d = INV2*g_i*g_j - 0.5 comes straight out of a K=2
  PE outer product (row1 of lhsT/rhs is a const -0.5/ones pair); DVE
  does same*d and relu(d)-same*d with accum_out. Diagonal terms run on
  ACT (Square then Relu(-x) with accum_out) in parallel.
"""

import numpy as np

import concourse.bacc as bacc
import concourse.mybir as mybir
import concourse.tile as tile
from concourse.bass_utils import run_bass_kernel_spmd

N_CORES = 8
B = 256
N = 512
BS = B // N_CORES          # 32 graphs per core
NN = N * N                 # 262144 elements per graph
P = 128                    # SBUF partitions
MARGIN = 0.5
NUM_PAIRS = B * (B - 1) // 2
INV2 = 1.0 / (float(NN) * float(NN))   # folds the two mean divisions
KK = 0.5 / NUM_PAIRS

# load-tile taper: graphs per tile (sum = 32)
TILE_G = [4, 4, 4, 4, 4, 4, 2, 2, 1, 1, 1, 1]
TILE_BASE = np.cumsum([0] + TILE_G).tolist()   # first graph per tile

# knobs for test.py (harness never touches these)
TRACE = False
TRACE_DIR = None
TRACE_CORES = None
LAST_EXEC_NS = None
LAST_RESULTS = None

_CACHED_NC = None


def build_body(tc, loss_ap, graph_ap, labels_ap, consts_ap):
    """Emit the per-core program. graph_ap: [BS, N, N] f32 shard,
    labels_ap: [1, B] f32 full labels, consts_ap: [P, 6] f32 block
    indicators (blk4 | blk2), loss_ap: [1, 1] f32 out."""
    nc = tc.nc
    f32 = mybir.dt.float32
    X = mybir.AxisListType.X
    ALU = mybir.AluOpType
    Copy = mybir.ActivationFunctionType.Copy

    flat = graph_ap.rearrange("b n m -> (b n m)")

    def gview(t):
        C = TILE_G[t] * 2048
        base = TILE_BASE[t] * NN
        return flat[base : base + P * C].rearrange("(p c) -> p c", c=C)

    with (
        tc.tile_pool(name="io4", bufs=4) as io4,     # [P, 8192] 32K/part
        tc.tile_pool(name="io2", bufs=2) as io2,     # [P, 4096] 16K/part
        tc.tile_pool(name="io1", bufs=3) as io1,     # [P, 2048]  8K/part
        tc.tile_pool(name="acc", bufs=1) as acc,
        tc.tile_pool(name="ps1", bufs=2, space="PSUM") as ps1,
        tc.tile_pool(name="psg", bufs=2, space="PSUM") as psg,
        tc.tile_pool(name="psd", bufs=2, space="PSUM") as psd,
        tc.tile_pool(name="dram", bufs=1, space="DRAM") as dram,
    ):
        # collective buffers (Shared outputs: HBM-HBM AllGather fast path)
        warm_in0 = dram.tile([1, 8], f32, tag="warm_in0")
        warm_out0 = dram.tile([N_CORES, 8], f32, tag="warm_out0", addr_space="Shared")
        cc_in = dram.tile([1, BS], f32, tag="cc_in")
        cc_out = dram.tile([N_CORES, BS], f32, tag="cc_out", addr_space="Shared")

        # ---- gpsimd (SWDGE) aux stream: warm-up CC first, then inputs ----
        nc.gpsimd.dma_start(warm_in0[:], labels_ap[:, 0:8])
        # warm-up collective, fired ~3us in: absorbs the CC entry barrier +
        # ncfw wakeup + rank launch skew so the real AllGather is ~1-2us.
        nc.gpsimd.collective_compute(
            "AllGather",
            ALU.bypass,
            replica_groups=[list(range(N_CORES))],
            ins=[warm_in0[:]],
            outs=[warm_out0[:]],
        )
        lab_row = acc.tile([1, B], f32, tag="lab_row")
        nc.gpsimd.dma_start(lab_row[:], labels_ap)
        lab_cols = []
        for c in range(2):
            lab_col = acc.tile([P, 1], f32, tag=f"lab_col{c}")
            nc.gpsimd.dma_start(lab_col[:], labels_ap[0, c * P : (c + 1) * P])
            lab_cols.append(lab_col)
        blk = acc.tile([P, 6], f32, tag="blk")       # blk4 | blk2
        nc.gpsimd.dma_start(blk[:], consts_ap)
        lb = acc.tile([P, B], f32, tag="lb")
        nc.gpsimd.partition_broadcast(lb[:], lab_row[:])

        # ---- DVE constants + same-label masks (early; waits only on lb) ----
        ones_col = acc.tile([P, 1], f32, tag="ones_col")
        nc.vector.memset(ones_col[:], 1.0)
        # pairwise-phase constant rows (row1 of the K=2 outer product)
        rhs2 = acc.tile([2, B], f32, tag="rhs2")      # row0: INV2*g, row1: 1
        nc.vector.memset(rhs2[:], 1.0)
        combo = acc.tile([2, B], f32, tag="combo")    # row0: raw g, row1: -0.5
        nc.vector.memset(combo[:], -MARGIN)           # lhsT chunks slice this
        diag_bias = acc.tile([1, 1], f32, tag="diag_bias")
        nc.vector.memset(diag_bias[:], MARGIN * KK)
        sames = []
        for c in range(2):
            same = acc.tile([P, B], f32, tag=f"same{c}")
            nc.vector.tensor_scalar(
                same[:], lb[:], lab_cols[c][:], None, ALU.is_equal
            )
            sames.append(same)

        # ---- heavy phase: tapered tiles on both rings ----
        # S columns: tile t -> col t (t = 0..10); tile 11 -> cols 11, 12
        S = acc.tile([P, 13], f32, tag="S")
        SQ = float(np.sqrt(INV2))
        # all 32 scaled per-graph sums accumulate into ONE SBUF row, then a
        # single 128 B cc_in write feeds the AllGather (four separate tiny
        # HBM writes cost ~1.5us completion EACH, serialized on the ring).
        cc_sb = acc.tile([1, BS], f32, tag="cc_sb")
        scrD = acc.tile([1, 2], f32, tag="scrD")

        pools = {4: io4, 2: io2, 1: io1}

        # Only per-engine emission order matters (engines run in-order;
        # cross-engine sync is via data-dep semaphores). Streams:
        #   sync: T0 T2 T4 T6 T8 T10 T11a ccdma graw loss
        #   ACT:  T1 T3 red0 T5 red2 T7 T9 red4 red6 red8 T11b red9 red10 red11a
        #   DVE:  consts sames red1 s0 s1 red3 s2 s3 red5 s4 s5 red7 s6..s9 red11b s10 s11
        # ACT issues every scalar-ring DMA before the reduce whose pool
        # slot it will need next, so the ring never stalls on compute.
        tl = {}
        for t in range(12):
            tl[t] = pools[TILE_G[t]].tile(
                [P, TILE_G[t] * 2048], f32,
                tag=f"g{TILE_G[t]}tile", name=f"gtile{t}",
            )

        def red_act(t, col=None):
            col = t if col is None else col
            nc.scalar.activation(
                tl[t][:], tl[t][:], Copy, accum_out=S[:, col : col + 1]
            )

        def red_dve(t, col=None):
            col = t if col is None else col
            nc.vector.reduce_sum(S[:, col : col + 1], tl[t][:], axis=X)

        def stage(t):
            # row-form per-graph sums: [1, G] = S[:, t]^T @ blk_G, scaled
            # into cc_sb[0, base:base+G] (identity graph order).
            G = TILE_G[t]
            base = TILE_BASE[t]
            rhs = ones_col[:] if G == 1 else blk[:, {4: 0, 2: 4}[G] : {4: 4, 2: 6}[G]]
            ps = psg.tile([1, G], f32, tag="psst", name=f"psr{t}")
            nc.tensor.matmul(ps[:], S[:, t : t + 1], rhs)
            nc.vector.tensor_scalar(
                cc_sb[0:1, base : base + G], ps[:], SQ, None, ALU.mult
            )

        H = 1024                                    # last-graph half cols
        nc.sync.dma_start(tl[0][:], gview(0))       # T0
        nc.scalar.dma_start(tl[1][:], gview(1))     # T1
        nc.scalar.dma_start(tl[3][:], gview(3))     # T3 (slot fresh)
        nc.sync.dma_start(tl[2][:], gview(2))       # T2
        red_act(0)                                  # ACT: t0 (sync ring)
        red_dve(1)                                  # DVE: t1
        stage(0)
        stage(1)
        nc.sync.dma_start(tl[4][:], gview(4))       # T4 (waits red0)
        nc.scalar.dma_start(tl[5][:], gview(5))     # T5 (waits red1)
        red_act(2)
        red_dve(3)
        stage(2)
        stage(3)
        nc.sync.dma_start(tl[6][:], gview(6))       # T6
        nc.scalar.dma_start(tl[7][:], gview(7))     # T7
        nc.scalar.dma_start(tl[9][:], gview(9))     # T9 (slot fresh)
        nc.sync.dma_start(tl[8][:], gview(8))       # T8
        nc.sync.dma_start(tl[10][:], gview(10))     # T10
        red_act(4)
        red_dve(5)
        stage(4)
        stage(5)
        red_act(6)
        red_dve(7)
        stage(6)
        stage(7)
        red_act(8)
        stage(8)
        # last graph (tile 11): halves on both rings, reduced in parallel
        nc.sync.dma_start(tl[11][:, 0:H], gview(11)[:, 0:H])     # T11a
        nc.scalar.dma_start(tl[11][:, H : 2 * H], gview(11)[:, H : 2 * H])
        red_act(9)    # t9 fits ACT's idle gap; keeps DVE free for the tail
        red_act(10)
        nc.scalar.activation(
            tl[11][:, 0:H], tl[11][:, 0:H], Copy, accum_out=S[:, 11:12]
        )
        nc.vector.reduce_sum(S[:, 12:13], tl[11][:, H : 2 * H], axis=X)
        stage(9)
        stage(10)
        # graph 31: cols 11+12 pair-merged via fused scale-and-add
        psD = psg.tile([1, 2], f32, tag="psst")
        nc.tensor.matmul(psD[:], ones_col[:], S[:, 11:13])
        nc.vector.tensor_scalar(
            scrD[:], psD[:], SQ, None, ALU.mult,
            op1=ALU.add, accum_out=cc_sb[0:1, 31:32],
        )
        nc.sync.dma_start(cc_in[:], cc_sb[:])

        # ---- all-gather the [BS] scaled sums -> [B] ----
        nc.gpsimd.collective_compute(
            "AllGather",
            ALU.bypass,
            replica_groups=[list(range(N_CORES))],
            ins=[cc_in[:]],
            outs=[cc_out[:]],
        )

        # gathered pre-scaled sums ghat = sqrt(INV2)*sum, flat [B] in DRAM;
        # two parallel DMAs fill both matmul operand rows, no scale op.
        flatg = cc_out[:].rearrange("r b -> (r b)")
        graw = combo[0:1, :]
        nc.sync.dma_start(graw, flatg[None, :])
        nc.scalar.dma_start(rhs2[0:1, :], flatg[None, :])

        # diagonal terms on ACT: dneg = sum(relu(0.5*KK - gg2)),
        # gg2 = (ghat*sqrt(KK))^2; loss uses -dneg.
        sq = float(np.sqrt(KK))
        gg2 = acc.tile([1, B], f32, tag="gg2")
        nc.scalar.activation(
            gg2[:], graw, mybir.ActivationFunctionType.Square, scale=sq
        )
        dneg = acc.tile([1, 1], f32, tag="dneg")
        relu_tmp = acc.tile([1, B], f32, tag="relu_tmp")
        nc.scalar.activation(
            relu_tmp[:], gg2[:], mybir.ActivationFunctionType.Relu,
            scale=-1.0, bias=diag_bias[:], accum_out=dneg[:],
        )

        # ---- pairwise loss: d straight out of a K=2 PE outer product ----
        # sum(relu(d) - same*d) split across engines: ACT accumulates
        # sum(relu(d)) (cols 0,1) while DVE accumulates sum(-same*d)
        # (cols 2,3) in parallel; all four columns then just add up.
        CS = acc.tile([P, 4], f32, tag="CS")
        for c in range(2):
            dps = psd.tile([P, B], f32, tag="dps")   # d = INV2*gi*gj - 0.5
            nc.tensor.matmul(dps[:], combo[:, c * P : (c + 1) * P], rhs2[:])
            rl = acc.tile([P, B], f32, tag=f"rl{c}")
            nc.scalar.activation(
                rl[:], dps[:], mybir.ActivationFunctionType.Relu,
                accum_out=CS[:, c : c + 1],
            )
            nsd = acc.tile([P, B], f32, tag=f"nsd{c}")   # -same * d
            nc.vector.scalar_tensor_tensor(
                nsd[:], dps[:], -1.0, sames[c][:], ALU.mult, ALU.mult,
                accum_out=CS[:, 2 + c : 3 + c],
            )

        # total = sum all (i,j); loss = KK*total - dneg
        ps_tot = ps1.tile([1, 4], f32, tag="ps_tot")
        nc.tensor.matmul(ps_tot[:], ones_col[:], CS[:])
        tk = acc.tile([1, 4], f32, tag="tk")
        totk = acc.tile([1, 1], f32, tag="totk")
        nc.vector.tensor_scalar(
            tk[:], ps_tot[:], KK, None, ALU.mult, op1=ALU.add,
            accum_out=totk[:],
        )
        res = acc.tile([1, 1], f32, tag="res")
        nc.vector.tensor_tensor(res[:], totk[:], dneg[:], ALU.subtract)
        nc.sync.dma_start(loss_ap, res[:])


def _consts_host():
    """[P, 6] f32 block indicators: blk4 (cols 0-3) | blk2 (cols 4-5)."""
    c = np.zeros((P, 6), dtype=np.float32)
    p = np.arange(P)
    for j in range(4):
        c[p // 32 == j, j] = 1.0
    for j in range(2):
        c[p // 64 == j, 4 + j] = 1.0
    return c


def _build():
    global _CACHED_NC
    if _CACHED_NC is not None:
        return _CACHED_NC
    nc = bacc.Bacc(
        "TRN2", target_bir_lowering=False, debug=False, num_devices=N_CORES
    )
    g_in = nc.dram_tensor(
        "graph", [BS, N, N], mybir.dt.float32, kind="ExternalInput"
    )
    lab_in = nc.dram_tensor(
        "labels_f32", [1, B], mybir.dt.float32, kind="ExternalInput"
    )
    consts_in = nc.dram_tensor(
        "consts", [P, 6], mybir.dt.float32, kind="ExternalInput"
    )
    out = nc.dram_tensor("loss", [1, 1], mybir.dt.float32, kind="ExternalOutput")
    with tile.TileContext(nc) as tc:
        build_body(tc, out.ap(), g_in.ap(), lab_in.ap(), consts_in.ap())
    nc.compile()
    _CACHED_NC = nc
    return nc


def kernel(graph, labels):
    global LAST_EXEC_NS, LAST_RESULTS
    graph = np.ascontiguousarray(np.asarray(graph), dtype=np.float32)
    labels_f32 = np.asarray(labels).astype(np.float32).reshape(1, B)
    assert graph.shape == (B, N, N)
    consts = _consts_host()

    nc = _build()
    in_maps = [
        {
            "graph": graph[c * BS : (c + 1) * BS],
            "labels_f32": labels_f32,
            "consts": consts,
        }
        for c in range(N_CORES)
    ]
    res = run_bass_kernel_spmd(
        nc,
        in_maps,
        core_ids=list(range(N_CORES)),
        trace=TRACE,
        tmpdir=TRACE_DIR,
        trace_cores=TRACE_CORES,
    )
    LAST_RESULTS = res
    LAST_EXEC_NS = res.exec_time_ns
    return np.asarray(res.results[0]["loss"][0, 0], dtype=np.float32)


# revision 9
# speedup vs baseline: 1.0690x; 1.0494x over previous
"""Bass/Trainium2 kernel for nn_ContrastiveAlignmentLoss.

reference math (B=256, N=512):
    global_sim = graph.mean(axis=(1, 2))                    # [B]
    sim        = outer(global_sim, global_sim)              # [B, B]
    same       = labels[:, None] == labels[None, :]
    pair_loss  = where(same, relu(0.5 - sim), relu(sim - 0.5))
    loss       = sum(triu(pair_loss, k=1)) / (B*(B-1)/2)

Distribution: data-parallel over B across 8 NeuronCores. Each core
sum-pools its 32 relation graphs (32 MiB/core of HBM reads - the whole
cost), AllGathers the tiny [B] scaled sums, and computes the pairwise
loss replicated. pair_loss is symmetric, so sum over i<j equals
(sum over all i,j - sum over diagonal) / 2, with
    pair(i,j) = relu(d) - same*d,   d = sim - 0.5
    pair(i,i) = relu(0.5 - g_i^2) = -min(g_i^2 - 0.5, 0)

Perf notes:
- the load is shaped around SDMA packet-processing overhead: with 1 MiB
  per-graph tiles every descriptor is an 8 KiB partition run and all 16
  SDMA engines saturate at ~400ns/packet (~303 GB/s aggregate). 4-graph
  tiles ([128, 8192]) make every descriptor a 32 KiB contiguous run,
  lifting per-engine throughput; graphs stay partition-aligned (each
  graph = 32 partitions), so per-graph sums come from a block-indicator
  matmul instead of ones - in graph order, no permutation anywhere.
- tile sizes taper 4,4,4,4,4,4,2,2,1,1,1,1 graphs so the reduces left
  after the last packet are short; the last graph is column-split
  across both rings with halves reduced on ACT+DVE in parallel.
- every auxiliary transfer (labels, consts, warm-up payload) rides the
  gpsimd SWDGE queue so the two HWDGE rings carry nothing but graph
  bytes from the first instruction on.
- engine streams are ordered to avoid in-order stalls: ACT issues each
  scalar-ring DMA before the (cross-ring) reduce that frees the next
  pool slot it will need; DVE (which issues nothing) takes the reduces
  that would otherwise couple a ring to its own data.
- a 32-byte warm-up AllGather fired ~3us in re-synchronizes the ranks
  so the real AllGather's ncfw wakeup + entry barrier cost ~1-2us
  instead of 20-60us.
- pairwise phase: d = INV2*g_i*g_j - 0.5 comes straight out of a K=2
  PE outer product (row1 of lhsT/rhs is a const -0.5/ones pair); DVE
  does same*d and relu(d)-same*d with accum_out. Diagonal terms run on
  ACT (Square then Relu(-x) with accum_out) in parallel.
"""

import numpy as np

import concourse.bacc as bacc
import concourse.mybir as mybir
import concourse.tile as tile
from concourse.bass_utils import run_bass_kernel_spmd

N_CORES = 8
B = 256
N = 512
BS = B // N_CORES          # 32 graphs per core
NN = N * N                 # 262144 elements per graph
P = 128                    # SBUF partitions
MARGIN = 0.5
NUM_PAIRS = B * (B - 1) // 2
INV2 = 1.0 / (float(NN) * float(NN))   # folds the two mean divisions
KK = 0.5 / NUM_PAIRS

# load-tile taper: graphs per tile (sum = 32)
TILE_G = [4, 4, 4, 4, 4, 4, 2, 2, 1, 1, 1, 1]
TILE_BASE = np.cumsum([0] + TILE_G).tolist()   # first graph per tile

# knobs for test.py (harness never touches these)
TRACE = False
TRACE_DIR = None
TRACE_CORES = None
LAST_EXEC_NS = None
LAST_RESULTS = None

_CACHED_NC = None


def build_body(tc, loss_ap, graph_ap, labels_ap, consts_ap):
    """Emit the per-core program. graph_ap: [BS, N, N] f32 shard,
    labels_ap: [1, B] f32 full labels, consts_ap: [P, 6] f32 block
    indicators (blk4 | blk2), loss_ap: [1, 1] f32 out."""
    nc = tc.nc
    f32 = mybir.dt.float32
    X = mybir.AxisListType.X
    ALU = mybir.AluOpType
    Copy = mybir.ActivationFunctionType.Copy

    flat = graph_ap.rearrange("b n m -> (b n m)")

    def gview(t):
        C = TILE_G[t] * 2048
        base = TILE_BASE[t] * NN
        return flat[base : base + P * C].rearrange("(p c) -> p c", c=C)

    with (
        tc.tile_pool(name="io4", bufs=4) as io4,     # [P, 8192] 32K/part
        tc.tile_pool(name="io2", bufs=2) as io2,     # [P, 4096] 16K/part
        tc.tile_pool(name="io1", bufs=3) as io1,     # [P, 2048]  8K/part
        tc.tile_pool(name="acc", bufs=1) as acc,
        tc.tile_pool(name="ps1", bufs=2, space="PSUM") as ps1,
        tc.tile_pool(name="psg", bufs=2, space="PSUM") as psg,
        tc.tile_pool(name="psd", bufs=2, space="PSUM") as psd,
        tc.tile_pool(name="dram", bufs=1, space="DRAM") as dram,
    ):
        # collective buffers (Shared outputs: HBM-HBM AllGather fast path)
        warm_in0 = dram.tile([1, 8], f32, tag="warm_in0")
        warm_out0 = dram.tile([N_CORES, 8], f32, tag="warm_out0", addr_space="Shared")
        cc_in = dram.tile([1, BS], f32, tag="cc_in")
        cc_out = dram.tile([N_CORES, BS], f32, tag="cc_out", addr_space="Shared")

        # ---- gpsimd (SWDGE) aux stream: warm-up CC first, then inputs ----
        nc.gpsimd.dma_start(warm_in0[:], labels_ap[:, 0:8])
        # warm-up collective, fired ~3us in: absorbs the CC entry barrier +
        # ncfw wakeup + rank launch skew so the real AllGather is ~1-2us.
        nc.gpsimd.collective_compute(
            "AllGather",
            ALU.bypass,
            replica_groups=[list(range(N_CORES))],
            ins=[warm_in0[:]],
            outs=[warm_out0[:]],
        )
        lab_row = acc.tile([1, B], f32, tag="lab_row")
        nc.gpsimd.dma_start(lab_row[:], labels_ap)
        lab_cols = []
        for c in range(2):
            lab_col = acc.tile([P, 1], f32, tag=f"lab_col{c}")
            nc.gpsimd.dma_start(lab_col[:], labels_ap[0, c * P : (c + 1) * P])
            lab_cols.append(lab_col)
        blk = acc.tile([P, 6], f32, tag="blk")       # blk4 | blk2
        nc.gpsimd.dma_start(blk[:], consts_ap)
        lb = acc.tile([P, B], f32, tag="lb")
        nc.gpsimd.partition_broadcast(lb[:], lab_row[:])

        # ---- DVE constants + same-label masks (early; waits only on lb) ----
        ones_col = acc.tile([P, 1], f32, tag="ones_col")
        nc.vector.memset(ones_col[:], 1.0)
        # pairwise-phase constant rows (row1 of the K=2 outer product)
        rhs2 = acc.tile([2, B], f32, tag="rhs2")      # row0: INV2*g, row1: 1
        nc.vector.memset(rhs2[:], 1.0)
        combo = acc.tile([2, B], f32, tag="combo")    # row0: raw g, row1: -0.5
        nc.vector.memset(combo[:], -MARGIN)           # lhsT chunks slice this
        diag_bias = acc.tile([1, 1], f32, tag="diag_bias")
        nc.vector.memset(diag_bias[:], MARGIN * KK)
        sames = []
        for c in range(2):
            same = acc.tile([P, B], f32, tag=f"same{c}")
            nc.vector.tensor_scalar(
                same[:], lb[:], lab_cols[c][:], None, ALU.is_equal
            )
            sames.append(same)

        # ---- heavy phase: tapered tiles on both rings ----
        # S columns: tile t -> col t (t = 0..10); tile 11 -> cols 11, 12
        S = acc.tile([P, 13], f32, tag="S")
        SQ = float(np.sqrt(INV2))
        # all 32 scaled per-graph sums accumulate into ONE SBUF row, then a
        # single 128 B cc_in write feeds the AllGather (four separate tiny
        # HBM writes cost ~1.5us completion EACH, serialized on the ring).
        cc_sb = acc.tile([1, BS], f32, tag="cc_sb")
        scrD = acc.tile([1, 2], f32, tag="scrD")

        pools = {4: io4, 2: io2, 1: io1}

        # Only per-engine emission order matters (engines run in-order;
        # cross-engine sync is via data-dep semaphores). Streams:
        #   sync: T0 T2 T4 T6 T8 T10 T11a ccdma graw loss
        #   ACT:  T1 T3 red0 T5 red2 T7 T9 red4 red6 red8 T11b red9 red10 red11a
        #   DVE:  consts sames red1 s0 s1 red3 s2 s3 red5 s4 s5 red7 s6..s9 red11b s10 s11
        # ACT issues every scalar-ring DMA before the reduce whose pool
        # slot it will need next, so the ring never stalls on compute.
        tl = {}
        for t in range(12):
            tl[t] = pools[TILE_G[t]].tile(
                [P, TILE_G[t] * 2048], f32,
                tag=f"g{TILE_G[t]}tile", name=f"gtile{t}",
            )

        def red_act(t, col=None):
            col = t if col is None else col
            nc.scalar.activation(
                tl[t][:], tl[t][:], Copy, accum_out=S[:, col : col + 1]
            )

        def red_dve(t, col=None):
            col = t if col is None else col
            nc.vector.reduce_sum(S[:, col : col + 1], tl[t][:], axis=X)

        def stage(t):
            # row-form per-graph sums: [1, G] = S[:, t]^T @ blk_G, scaled
            # into cc_sb[0, base:base+G] (identity graph order).
            G = TILE_G[t]
            base = TILE_BASE[t]
            rhs = ones_col[:] if G == 1 else blk[:, {4: 0, 2: 4}[G] : {4: 4, 2: 6}[G]]
            ps = psg.tile([1, G], f32, tag="psst", name=f"psr{t}")
            nc.tensor.matmul(ps[:], S[:, t : t + 1], rhs)
            nc.vector.tensor_scalar(
                cc_sb[0:1, base : base + G], ps[:], SQ, None, ALU.mult
            )

        H = 1024                                    # last-graph half cols
        nc.sync.dma_start(tl[0][:], gview(0))       # T0
        nc.scalar.dma_start(tl[1][:], gview(1))     # T1
        nc.scalar.dma_start(tl[3][:], gview(3))     # T3 (slot fresh)
        nc.sync.dma_start(tl[2][:], gview(2))       # T2
        red_act(0)                                  # ACT: t0 (sync ring)
        red_dve(1)                                  # DVE: t1
        stage(0)
        stage(1)
        nc.sync.dma_start(tl[4][:], gview(4))       # T4 (waits red0)
        nc.scalar.dma_start(tl[5][:], gview(5))     # T5 (waits red1)
        red_act(2)
        red_dve(3)
        stage(2)
        stage(3)
        nc.sync.dma_start(tl[6][:], gview(6))       # T6
        nc.scalar.dma_start(tl[7][:], gview(7))     # T7
        nc.scalar.dma_start(tl[9][:], gview(9))     # T9 (slot fresh)
        nc.sync.dma_start(tl[8][:], gview(8))       # T8
        nc.sync.dma_start(tl[10][:], gview(10))     # T10
        red_act(4)
        red_dve(5)
        stage(4)
        stage(5)
        red_act(6)
        red_dve(7)
        stage(6)
        stage(7)
        red_act(8)
        stage(8)
        # last graph (tile 11): halves on both rings, reduced in parallel
        nc.sync.dma_start(tl[11][:, 0:H], gview(11)[:, 0:H])     # T11a
        nc.scalar.dma_start(tl[11][:, H : 2 * H], gview(11)[:, H : 2 * H])
        red_act(9)    # t9 fits ACT's idle gap; keeps DVE free for the tail
        red_act(10)
        nc.scalar.activation(
            tl[11][:, 0:H], tl[11][:, 0:H], Copy, accum_out=S[:, 11:12]
        )
        nc.vector.reduce_sum(S[:, 12:13], tl[11][:, H : 2 * H], axis=X)
        stage(9)
        stage(10)
        # graph 31: cols 11+12 pair-merged via fused scale-and-add
        psD = psg.tile([1, 2], f32, tag="psst")
        nc.tensor.matmul(psD[:], ones_col[:], S[:, 11:13])
        nc.vector.tensor_scalar(
            scrD[:], psD[:], SQ, None, ALU.mult,
            op1=ALU.add, accum_out=cc_sb[0:1, 31:32],
        )
        nc.sync.dma_start(cc_in[:], cc_sb[:])

        # ---- all-gather the [BS] scaled sums -> [B] ----
        nc.gpsimd.collective_compute(
            "AllGather",
            ALU.bypass,
            replica_groups=[list(range(N_CORES))],
            ins=[cc_in[:]],
            outs=[cc_out[:]],
        )

        # gathered pre-scaled sums ghat = sqrt(INV2)*sum, flat [B] in DRAM;
        # two parallel DMAs fill both matmul operand rows, no scale op.
        flatg = cc_out[:].rearrange("r b -> (r b)")
        graw = combo[0:1, :]
        nc.sync.dma_start(graw, flatg[None, :])
        nc.scalar.dma_start(rhs2[0:1, :], flatg[None, :])

        # diagonal terms on ACT: dneg = sum(relu(0.5*KK - gg2)),
        # gg2 = (ghat*sqrt(KK))^2; loss uses -dneg.
        sq = float(np.sqrt(KK))
        gg2 = acc.tile([1, B], f32, tag="gg2")
        nc.scalar.activation(
            gg2[:], graw, mybir.ActivationFunctionType.Square, scale=sq
        )
        dneg = acc.tile([1, 1], f32, tag="dneg")
        relu_tmp = acc.tile([1, B], f32, tag="relu_tmp")
        nc.scalar.activation(
            relu_tmp[:], gg2[:], mybir.ActivationFunctionType.Relu,
            scale=-1.0, bias=diag_bias[:], accum_out=dneg[:],
        )

        # ---- pairwise loss: d straight out of a K=2 PE outer product ----
        # sum(relu(d) - same*d) split across engines: ACT accumulates
        # sum(relu(d)) into CSr while DVE accumulates sum(-same*d) into
        # CSn in parallel (separate tiles - a shared accum tile's writer
        # tracking would serialize the engines); all 4 columns add up.
        CSr = acc.tile([P, 2], f32, tag="CSr")
        CSn = acc.tile([P, 2], f32, tag="CSn")
        for c in range(2):
            dps = psd.tile([P, B], f32, tag="dps")   # d = INV2*gi*gj - 0.5
            nc.tensor.matmul(dps[:], combo[:, c * P : (c + 1) * P], rhs2[:])
            rl = acc.tile([P, B], f32, tag=f"rl{c}")
            nc.scalar.activation(
                rl[:], dps[:], mybir.ActivationFunctionType.Relu,
                accum_out=CSr[:, c : c + 1],
            )
            nsd = acc.tile([P, B], f32, tag=f"nsd{c}")   # -same * d
            nc.vector.scalar_tensor_tensor(
                nsd[:], dps[:], -1.0, sames[c][:], ALU.mult, ALU.mult,
                accum_out=CSn[:, c : c + 1],
            )

        # total = sum all (i,j); loss = KK*total - dneg
        ps_tot = ps1.tile([1, 4], f32, tag="ps_tot")
        nc.tensor.matmul(ps_tot[:, 0:2], ones_col[:], CSr[:])
        nc.tensor.matmul(ps_tot[:, 2:4], ones_col[:], CSn[:])
        tk = acc.tile([1, 4], f32, tag="tk")
        totk = acc.tile([1, 1], f32, tag="totk")
        nc.vector.tensor_scalar(
            tk[:], ps_tot[:], KK, None, ALU.mult, op1=ALU.add,
            accum_out=totk[:],
        )
        res = acc.tile([1, 1], f32, tag="res")
        nc.vector.tensor_tensor(res[:], totk[:], dneg[:], ALU.subtract)
        nc.sync.dma_start(loss_ap, res[:])


def _consts_host():
    """[P, 6] f32 block indicators: blk4 (cols 0-3) | blk2 (cols 4-5)."""
    c = np.zeros((P, 6), dtype=np.float32)
    p = np.arange(P)
    for j in range(4):
        c[p // 32 == j, j] = 1.0
    for j in range(2):
        c[p // 64 == j, 4 + j] = 1.0
    return c


def _build():
    global _CACHED_NC
    if _CACHED_NC is not None:
        return _CACHED_NC
    nc = bacc.Bacc(
        "TRN2", target_bir_lowering=False, debug=False, num_devices=N_CORES
    )
    g_in = nc.dram_tensor(
        "graph", [BS, N, N], mybir.dt.float32, kind="ExternalInput"
    )
    lab_in = nc.dram_tensor(
        "labels_f32", [1, B], mybir.dt.float32, kind="ExternalInput"
    )
    consts_in = nc.dram_tensor(
        "consts", [P, 6], mybir.dt.float32, kind="ExternalInput"
    )
    out = nc.dram_tensor("loss", [1, 1], mybir.dt.float32, kind="ExternalOutput")
    with tile.TileContext(nc) as tc:
        build_body(tc, out.ap(), g_in.ap(), lab_in.ap(), consts_in.ap())
    nc.compile()
    _CACHED_NC = nc
    return nc


def kernel(graph, labels):
    global LAST_EXEC_NS, LAST_RESULTS
    graph = np.ascontiguousarray(np.asarray(graph), dtype=np.float32)
    labels_f32 = np.asarray(labels).astype(np.float32).reshape(1, B)
    assert graph.shape == (B, N, N)
    consts = _consts_host()

    nc = _build()
    in_maps = [
        {
            "graph": graph[c * BS : (c + 1) * BS],
            "labels_f32": labels_f32,
            "consts": consts,
        }
        for c in range(N_CORES)
    ]
    res = run_bass_kernel_spmd(
        nc,
        in_maps,
        core_ids=list(range(N_CORES)),
        trace=TRACE,
        tmpdir=TRACE_DIR,
        trace_cores=TRACE_CORES,
    )
    LAST_RESULTS = res
    LAST_EXEC_NS = res.exec_time_ns
    return np.asarray(res.results[0]["loss"][0, 0], dtype=np.float32)


# revision 11
# speedup vs baseline: 1.1208x; 1.0485x over previous
"""Bass/Trainium2 kernel for nn_ContrastiveAlignmentLoss.

reference math (B=256, N=512):
    global_sim = graph.mean(axis=(1, 2))                    # [B]
    sim        = outer(global_sim, global_sim)              # [B, B]
    same       = labels[:, None] == labels[None, :]
    pair_loss  = where(same, relu(0.5 - sim), relu(sim - 0.5))
    loss       = sum(triu(pair_loss, k=1)) / (B*(B-1)/2)

Distribution: data-parallel over B across 8 NeuronCores. Each core
sum-pools its 32 relation graphs (32 MiB/core of HBM reads - the whole
cost), AllGathers the tiny [B] scaled sums, and computes the pairwise
loss replicated. pair_loss is symmetric, so sum over i<j equals
(sum over all i,j - sum over diagonal) / 2, with
    pair(i,j) = relu(d) - same*d,   d = sim - 0.5
    pair(i,i) = relu(0.5 - g_i^2) = -min(g_i^2 - 0.5, 0)

Perf notes:
- the load is shaped around SDMA packet-processing overhead: with 1 MiB
  per-graph tiles every descriptor is an 8 KiB partition run and all 16
  SDMA engines saturate at ~400ns/packet (~303 GB/s aggregate). 4-graph
  tiles ([128, 8192]) make every descriptor a 32 KiB contiguous run,
  lifting per-engine throughput; graphs stay partition-aligned (each
  graph = 32 partitions), so per-graph sums come from a block-indicator
  matmul instead of ones - in graph order, no permutation anywhere.
- tile sizes taper 4,4,4,4,4,4,2,2,1,1,1,1 graphs so the reduces left
  after the last packet are short; the last graph is column-split
  across both rings with halves reduced on ACT+DVE in parallel.
- every auxiliary transfer (labels, consts, warm-up payload) rides the
  gpsimd SWDGE queue so the two HWDGE rings carry nothing but graph
  bytes from the first instruction on.
- engine streams are ordered to avoid in-order stalls: ACT issues each
  scalar-ring DMA before the (cross-ring) reduce that frees the next
  pool slot it will need; DVE (which issues nothing) takes the reduces
  that would otherwise couple a ring to its own data.
- a 32-byte warm-up AllGather fired ~3us in re-synchronizes the ranks
  so the real AllGather's ncfw wakeup + entry barrier cost ~1-2us
  instead of 20-60us.
- pairwise phase: d = INV2*g_i*g_j - 0.5 comes straight out of a K=2
  PE outer product (row1 of lhsT/rhs is a const -0.5/ones pair); DVE
  does same*d and relu(d)-same*d with accum_out. Diagonal terms run on
  ACT (Square then Relu(-x) with accum_out) in parallel.
"""

import numpy as np

import concourse.bacc as bacc
import concourse.mybir as mybir
import concourse.tile as tile
from concourse.bass_utils import run_bass_kernel_spmd

N_CORES = 8
B = 256
N = 512
BS = B // N_CORES          # 32 graphs per core
NN = N * N                 # 262144 elements per graph
P = 128                    # SBUF partitions
MARGIN = 0.5
NUM_PAIRS = B * (B - 1) // 2
INV2 = 1.0 / (float(NN) * float(NN))   # folds the two mean divisions
KK = 0.5 / NUM_PAIRS

# load-tile taper: graphs per tile (sum = 32)
TILE_G = [4, 4, 4, 4, 4, 4, 2, 2, 1, 1, 1, 1]
TILE_BASE = np.cumsum([0] + TILE_G).tolist()   # first graph per tile

# knobs for test.py (harness never touches these)
TRACE = False
TRACE_DIR = None
TRACE_CORES = None
LAST_EXEC_NS = None
LAST_RESULTS = None

_CACHED_NC = None


def build_body(tc, loss_ap, graph_ap, labels_ap, consts_ap):
    """Emit the per-core program. graph_ap: [BS, N, N] f32 shard,
    labels_ap: [1, B] f32 full labels, consts_ap: [P, 6] f32 block
    indicators (blk4 | blk2), loss_ap: [1, 1] f32 out."""
    nc = tc.nc
    f32 = mybir.dt.float32
    X = mybir.AxisListType.X
    ALU = mybir.AluOpType
    Copy = mybir.ActivationFunctionType.Copy

    flat = graph_ap.rearrange("b n m -> (b n m)")

    def gview(t):
        C = TILE_G[t] * 2048
        base = TILE_BASE[t] * NN
        return flat[base : base + P * C].rearrange("(p c) -> p c", c=C)

    with (
        tc.tile_pool(name="io4", bufs=4) as io4,     # [P, 8192] 32K/part
        tc.tile_pool(name="io2", bufs=2) as io2,     # [P, 4096] 16K/part
        tc.tile_pool(name="io1", bufs=3) as io1,     # [P, 2048]  8K/part
        tc.tile_pool(name="acc", bufs=1) as acc,
        tc.tile_pool(name="ps1", bufs=2, space="PSUM") as ps1,
        tc.tile_pool(name="psg", bufs=2, space="PSUM") as psg,
        tc.tile_pool(name="psd", bufs=2, space="PSUM") as psd,
        tc.tile_pool(name="dram", bufs=1, space="DRAM") as dram,
    ):
        # collective buffers (Shared outputs: HBM-HBM AllGather fast path)
        warm_in0 = dram.tile([1, 8], f32, tag="warm_in0")
        warm_out0 = dram.tile([N_CORES, 8], f32, tag="warm_out0", addr_space="Shared")
        cc_in = dram.tile([1, BS], f32, tag="cc_in")
        cc_out = dram.tile([N_CORES, BS], f32, tag="cc_out", addr_space="Shared")

        # ---- gpsimd (SWDGE) aux stream: warm-up CC first, then inputs ----
        nc.gpsimd.dma_start(warm_in0[:], labels_ap[:, 0:8])
        # warm-up collective, fired ~3us in: absorbs the CC entry barrier +
        # ncfw wakeup + rank launch skew so the real AllGather is ~1-2us.
        nc.gpsimd.collective_compute(
            "AllGather",
            ALU.bypass,
            replica_groups=[list(range(N_CORES))],
            ins=[warm_in0[:]],
            outs=[warm_out0[:]],
        )
        lab_row = acc.tile([1, B], f32, tag="lab_row")
        nc.gpsimd.dma_start(lab_row[:], labels_ap)
        lab_cols = []
        for c in range(2):
            lab_col = acc.tile([P, 1], f32, tag=f"lab_col{c}")
            nc.gpsimd.dma_start(lab_col[:], labels_ap[0, c * P : (c + 1) * P])
            lab_cols.append(lab_col)
        blk = acc.tile([P, 6], f32, tag="blk")       # blk4 | blk2
        nc.gpsimd.dma_start(blk[:], consts_ap)
        lb = acc.tile([P, B], f32, tag="lb")
        nc.gpsimd.partition_broadcast(lb[:], lab_row[:])

        # ---- DVE constants + same-label masks (early; waits only on lb) ----
        bf16 = mybir.dt.bfloat16
        ones_col = acc.tile([P, 1], f32, tag="ones_col")
        nc.vector.memset(ones_col[:], 1.0)
        # pairwise-phase constant rows (row1 of the K=2 outer product).
        # bf16 operands: the ghat values cluster at 0.5 so rounding error
        # is zero-mean and sub-1e-4 on the loss, and bf16 makes the PE
        # matmuls single-pass (~0.35us vs ~1.4us fp32 LOW+HIGH).
        rhs2 = acc.tile([2, B], bf16, tag="rhs2")     # row0: ghat, row1: 1
        nc.vector.memset(rhs2[:], 1.0)
        combo = acc.tile([2, B], bf16, tag="combo")   # row0: ghat, row1: -0.5
        nc.vector.memset(combo[:], -MARGIN)           # lhsT chunks slice this
        diag_bias = acc.tile([1, 1], f32, tag="diag_bias")
        nc.vector.memset(diag_bias[:], MARGIN * KK)
        sames = []
        for c in range(2):
            same = acc.tile([P, B], f32, tag=f"same{c}")
            nc.vector.tensor_scalar(
                same[:], lb[:], lab_cols[c][:], None, ALU.is_equal
            )
            sames.append(same)

        # ---- heavy phase: tapered tiles on both rings ----
        # S columns: tile t -> col t (t = 0..10); tile 11 -> cols 11, 12
        S = acc.tile([P, 13], f32, tag="S")
        SQ = float(np.sqrt(INV2))
        # all 32 scaled per-graph sums accumulate into ONE SBUF row, then a
        # single 128 B cc_in write feeds the AllGather (four separate tiny
        # HBM writes cost ~1.5us completion EACH, serialized on the ring).
        cc_sb = acc.tile([1, BS], f32, tag="cc_sb")
        scrD = acc.tile([1, 2], f32, tag="scrD")

        pools = {4: io4, 2: io2, 1: io1}

        # Only per-engine emission order matters (engines run in-order;
        # cross-engine sync is via data-dep semaphores). Streams:
        #   sync: T0 T2 T4 T6 T8 T10 T11a ccdma graw loss
        #   ACT:  T1 T3 red0 T5 red2 T7 T9 red4 red6 red8 T11b red9 red10 red11a
        #   DVE:  consts sames red1 s0 s1 red3 s2 s3 red5 s4 s5 red7 s6..s9 red11b s10 s11
        # ACT issues every scalar-ring DMA before the reduce whose pool
        # slot it will need next, so the ring never stalls on compute.
        tl = {}
        for t in range(12):
            tl[t] = pools[TILE_G[t]].tile(
                [P, TILE_G[t] * 2048], f32,
                tag=f"g{TILE_G[t]}tile", name=f"gtile{t}",
            )

        def red_act(t, col=None):
            col = t if col is None else col
            nc.scalar.activation(
                tl[t][:], tl[t][:], Copy, accum_out=S[:, col : col + 1]
            )

        def red_dve(t, col=None):
            col = t if col is None else col
            nc.vector.reduce_sum(S[:, col : col + 1], tl[t][:], axis=X)

        def stage(t):
            # row-form per-graph sums: [1, G] = S[:, t]^T @ blk_G, scaled
            # into cc_sb[0, base:base+G] (identity graph order).
            G = TILE_G[t]
            base = TILE_BASE[t]
            rhs = ones_col[:] if G == 1 else blk[:, {4: 0, 2: 4}[G] : {4: 4, 2: 6}[G]]
            ps = psg.tile([1, G], f32, tag="psst", name=f"psr{t}")
            nc.tensor.matmul(ps[:], S[:, t : t + 1], rhs)
            nc.vector.tensor_scalar(
                cc_sb[0:1, base : base + G], ps[:], SQ, None, ALU.mult
            )

        H = 1024                                    # last-graph half cols
        nc.sync.dma_start(tl[0][:], gview(0))       # T0
        nc.scalar.dma_start(tl[1][:], gview(1))     # T1
        nc.scalar.dma_start(tl[3][:], gview(3))     # T3 (slot fresh)
        nc.sync.dma_start(tl[2][:], gview(2))       # T2
        red_act(0)                                  # ACT: t0 (sync ring)
        red_dve(1)                                  # DVE: t1
        stage(0)
        stage(1)
        nc.sync.dma_start(tl[4][:], gview(4))       # T4 (waits red0)
        nc.scalar.dma_start(tl[5][:], gview(5))     # T5 (waits red1)
        red_act(2)
        red_dve(3)
        stage(2)
        stage(3)
        nc.sync.dma_start(tl[6][:], gview(6))       # T6
        nc.scalar.dma_start(tl[7][:], gview(7))     # T7
        nc.scalar.dma_start(tl[9][:], gview(9))     # T9 (slot fresh)
        nc.sync.dma_start(tl[8][:], gview(8))       # T8
        nc.sync.dma_start(tl[10][:], gview(10))     # T10
        red_act(4)
        red_dve(5)
        stage(4)
        stage(5)
        red_act(6)
        red_dve(7)
        stage(6)
        stage(7)
        red_act(8)
        stage(8)
        # last graph (tile 11): halves on both rings, reduced in parallel
        nc.sync.dma_start(tl[11][:, 0:H], gview(11)[:, 0:H])     # T11a
        nc.scalar.dma_start(tl[11][:, H : 2 * H], gview(11)[:, H : 2 * H])
        red_act(9)    # t9 fits ACT's idle gap; keeps DVE free for the tail
        red_act(10)
        nc.scalar.activation(
            tl[11][:, 0:H], tl[11][:, 0:H], Copy, accum_out=S[:, 11:12]
        )
        nc.vector.reduce_sum(S[:, 12:13], tl[11][:, H : 2 * H], axis=X)
        stage(9)
        stage(10)
        # graph 31: cols 11+12 pair-merged via fused scale-and-add
        psD = psg.tile([1, 2], f32, tag="psst")
        nc.tensor.matmul(psD[:], ones_col[:], S[:, 11:13])
        nc.vector.tensor_scalar(
            scrD[:], psD[:], SQ, None, ALU.mult,
            op1=ALU.add, accum_out=cc_sb[0:1, 31:32],
        )
        nc.sync.dma_start(cc_in[:], cc_sb[:])

        # ---- all-gather the [BS] scaled sums -> [B] ----
        nc.gpsimd.collective_compute(
            "AllGather",
            ALU.bypass,
            replica_groups=[list(range(N_CORES))],
            ins=[cc_in[:]],
            outs=[cc_out[:]],
        )

        # gathered pre-scaled sums ghat = sqrt(INV2)*sum, flat [B] in DRAM;
        # one DMA into an f32 row, then on-chip casts fill both bf16
        # matmul operand rows (cheaper than a second HBM gather).
        flatg = cc_out[:].rearrange("r b -> (r b)")
        gf = acc.tile([1, B], f32, tag="gf")
        nc.sync.dma_start(gf[:], flatg[None, :])
        nc.vector.tensor_scalar(combo[0:1, :], gf[:], 1.0, None, ALU.mult)
        nc.scalar.activation(rhs2[0:1, :], gf[:], Copy)

        # diagonal terms on ACT: dneg = sum(relu(0.5*KK - gg2)),
        # gg2 = (ghat*sqrt(KK))^2; loss uses -dneg (full f32 from gf).
        sq = float(np.sqrt(KK))
        gg2 = acc.tile([1, B], f32, tag="gg2")
        nc.scalar.activation(
            gg2[:], gf[:], mybir.ActivationFunctionType.Square, scale=sq
        )
        dneg = acc.tile([1, 1], f32, tag="dneg")
        relu_tmp = acc.tile([1, B], f32, tag="relu_tmp")
        nc.scalar.activation(
            relu_tmp[:], gg2[:], mybir.ActivationFunctionType.Relu,
            scale=-1.0, bias=diag_bias[:], accum_out=dneg[:],
        )

        # ---- pairwise loss: d straight out of a K=2 PE outer product ----
        # sum(relu(d) - same*d) split across engines: ACT accumulates
        # sum(relu(d)) into CSr while DVE accumulates sum(-same*d) into
        # CSn in parallel (separate tiles - a shared accum tile's writer
        # tracking would serialize the engines); all 4 columns add up.
        CSr = acc.tile([P, 2], f32, tag="CSr")
        CSn = acc.tile([P, 2], f32, tag="CSn")
        for c in range(2):
            dps = psd.tile([P, B], f32, tag="dps")   # d = INV2*gi*gj - 0.5
            nc.tensor.matmul(dps[:], combo[:, c * P : (c + 1) * P], rhs2[:])
            rl = acc.tile([P, B], f32, tag=f"rl{c}")
            nc.scalar.activation(
                rl[:], dps[:], mybir.ActivationFunctionType.Relu,
                accum_out=CSr[:, c : c + 1],
            )
            nsd = acc.tile([P, B], f32, tag=f"nsd{c}")   # -same * d
            nc.vector.scalar_tensor_tensor(
                nsd[:], dps[:], -1.0, sames[c][:], ALU.mult, ALU.mult,
                accum_out=CSn[:, c : c + 1],
            )

        # total = sum all (i,j); loss = KK*total - dneg
        ps_tot = ps1.tile([1, 4], f32, tag="ps_tot")
        nc.tensor.matmul(ps_tot[:, 0:2], ones_col[:], CSr[:])
        nc.tensor.matmul(ps_tot[:, 2:4], ones_col[:], CSn[:])
        tk = acc.tile([1, 4], f32, tag="tk")
        totk = acc.tile([1, 1], f32, tag="totk")
        nc.vector.tensor_scalar(
            tk[:], ps_tot[:], KK, None, ALU.mult, op1=ALU.add,
            accum_out=totk[:],
        )
        res = acc.tile([1, 1], f32, tag="res")
        nc.vector.tensor_tensor(res[:], totk[:], dneg[:], ALU.subtract)
        nc.sync.dma_start(loss_ap, res[:])


def _consts_host():
    """[P, 6] f32 block indicators: blk4 (cols 0-3) | blk2 (cols 4-5)."""
    c = np.zeros((P, 6), dtype=np.float32)
    p = np.arange(P)
    for j in range(4):
        c[p // 32 == j, j] = 1.0
    for j in range(2):
        c[p // 64 == j, 4 + j] = 1.0
    return c


def _build():
    global _CACHED_NC
    if _CACHED_NC is not None:
        return _CACHED_NC
    nc = bacc.Bacc(
        "TRN2", target_bir_lowering=False, debug=False, num_devices=N_CORES
    )
    g_in = nc.dram_tensor(
        "graph", [BS, N, N], mybir.dt.float32, kind="ExternalInput"
    )
    lab_in = nc.dram_tensor(
        "labels_f32", [1, B], mybir.dt.float32, kind="ExternalInput"
    )
    consts_in = nc.dram_tensor(
        "consts", [P, 6], mybir.dt.float32, kind="ExternalInput"
    )
    out = nc.dram_tensor("loss", [1, 1], mybir.dt.float32, kind="ExternalOutput")
    with tile.TileContext(nc) as tc:
        build_body(tc, out.ap(), g_in.ap(), lab_in.ap(), consts_in.ap())
    nc.compile()
    _CACHED_NC = nc
    return nc


def kernel(graph, labels):
    global LAST_EXEC_NS, LAST_RESULTS
    graph = np.ascontiguousarray(np.asarray(graph), dtype=np.float32)
    labels_f32 = np.asarray(labels).astype(np.float32).reshape(1, B)
    assert graph.shape == (B, N, N)
    consts = _consts_host()

    nc = _build()
    in_maps = [
        {
            "graph": graph[c * BS : (c + 1) * BS],
            "labels_f32": labels_f32,
            "consts": consts,
        }
        for c in range(N_CORES)
    ]
    res = run_bass_kernel_spmd(
        nc,
        in_maps,
        core_ids=list(range(N_CORES)),
        trace=TRACE,
        tmpdir=TRACE_DIR,
        trace_cores=TRACE_CORES,
    )
    LAST_RESULTS = res
    LAST_EXEC_NS = res.exec_time_ns
    return np.asarray(res.results[0]["loss"][0, 0], dtype=np.float32)
